# revision 1
# baseline (speedup 1.0000x reference)
"""BoundaryLoss Trainium2 kernel (8 NeuronCores, SPMD).

Pipeline (per core c):
  1. Row pass on the core's 128-row block of each image: 1D nearest-background
     distance via two tensor_tensor_scan ops (forward/reverse recurrence
     state = min(state+1, z)), square -> g2.
  2. PE-transpose g2 into 128x128 blocks, AllToAll so core c ends up with
     g2^T for column block c over all 1024 source rows (both images).
  3. Column min-plus pass D2[j,i] = min_dd (dd^2 + g2T[j, i+dd]) over a
     window dd in [-W, W] on the Vector engine. W is chosen on the host per
     image as the max row-distance (exact bound: a source row further than
     g[i,j] cannot win since (i-k)^2 > g2[i,j] >= D2[i,j]), rounded up to a
     bucket. When W <= 15 every candidate that can win is a small integer
     that bf16 represents exactly, so the chain runs in bf16 using plain
     TT(2x)/TS(4x) ops (the fused STT has no fast uop); odd shifts read a
     one-element-shifted copy to keep 4-byte alignment for the 2x mode.
     A dummy AllReduce issued at t=0 absorbs this runtime's ~80us
     first-collective-of-the-execution latency floor under the row pass.
  4. sqrt (ACT, <=7e-6 rel err, exact at 0), global max via AllReduce,
     normalize, boundary mask, masked |diff| partial sums; the host sums
     the 8 partial pairs and divides.
"""
import os
import sys

import numpy as np

for _p in ("/opt/trn_rl_repo", "/root/.axon_site/_ro/trn_rl_repo"):
    if os.path.isdir(_p) and _p not in sys.path:
        sys.path.append(_p)

import concourse.bacc as bacc
import concourse.tile as tile
from concourse import mybir
from concourse.bass_utils import run_bass_kernel_spmd

F32 = mybir.dt.float32
BF16 = mybir.dt.bfloat16
I32 = mybir.dt.int32
AF = mybir.ActivationFunctionType
ALU = mybir.AluOpType
AX = mybir.AxisListType

H = 1024          # image height/width
P = 128           # partitions / rows per core / cols per j-block
NCORES = 8
BIG = 1.0e4
INF = 1.0e9
BF16_GMAX = 15    # bf16 exact iff winners (<= gmax^2) stay <= 255

_BUCKETS = (8, 12, 16, 20, 24, 32, 40, 48, 64, 96, 128, 192, 256, 384, 512,
            768, 1023)


def _col_pass(tc, m, w, use_bf16, a2a_out, bases, persist, work):
    """Windowed min-plus for image m; returns acc tile [P, H] (f32 or bf16).

    acc[j, i] = min_{|dd| <= w} (dd^2 + g2T[j, i+dd]), INF-padded outside
    the column range. Entirely on the Vector engine (this compiler build
    rejects tensor ops on Pool).
    """
    nc = tc.nc
    gw = H + 2 * w
    dt = BF16 if use_bf16 else F32
    gTp = persist.tile([P, gw], dt, tag=f"gtp{m}")
    nc.vector.memset(gTp[:, :w], INF)
    nc.vector.memset(gTp[:, w + H:], INF)
    for r in range(NCORES):
        base = bases[r]
        nc.sync.dma_start(gTp[:, w + r * P:w + (r + 1) * P],
                          a2a_out[base:base + P, :])
    if use_bf16:
        # odd shifts read a one-element-shifted copy so the AP stays
        # 4-byte-aligned for the DVE 2x bf16 mode
        gB = persist.tile([P, gw], BF16, tag=f"gb{m}")
        nc.vector.tensor_copy(gB[:, :gw - 1], gTp[:, 1:])
        nc.vector.memset(gB[:, gw - 1:], INF)

        def shifted(off):  # AP of width H at element offset `off` of gTp
            if off % 2 == 0:
                return gTp[:, off:off + H]
            return gB[:, off - 1:off - 1 + H]
        acc_t = BF16
    else:
        def shifted(off):
            return gTp[:, off:off + H]
        acc_t = F32

    acc = persist.tile([P, H], acc_t, tag=f"acc{m}")
    # Pool (GpSimd) is restricted to memset/iota/DMA/CC in this compiler
    # build, so the chain runs on DVE. The fused STT has no 2x uop (1213ns
    # regardless of dtype), while plain TT gets 2x and single-src TS gets
    # 4x in bf16 — so in bf16 a 3-op pairwise form is ~35% faster per dd.
    # dd=1 folds the d=0 term so no separate init copy is needed.
    if use_bf16:
        for dd in range(1, w + 1):
            tmp = work.tile([P, H], BF16, tag=f"pm{m}_{dd % 3}")
            nc.vector.tensor_tensor(tmp[:], shifted(w + dd), shifted(w - dd),
                                    ALU.min)
            nc.vector.tensor_scalar_add(tmp[:], tmp[:], float(dd * dd))
            nc.vector.tensor_tensor(
                acc[:], shifted(w) if dd == 1 else acc[:], tmp[:], ALU.min)
    else:
        for dd in range(1, w + 1):
            c = float(dd * dd)
            nc.vector.scalar_tensor_tensor(
                acc[:], shifted(w + dd), c,
                shifted(w) if dd == 1 else acc[:], ALU.add, ALU.min)
            nc.vector.scalar_tensor_tensor(
                acc[:], shifted(w - dd), c, acc[:], ALU.add, ALU.min)
    return acc


def _body(tc, w_gt, w_pred, bf_gt, bf_pred, gt_rows, pred_rows, partials):
    nc = tc.nc
    rg = [list(range(NCORES))]

    with tc.tile_pool(name="const", bufs=1) as const, \
         tc.tile_pool(name="work", bufs=2) as work, \
         tc.tile_pool(name="persist", bufs=1) as persist, \
         tc.tile_pool(name="ps", bufs=1, space="PSUM") as ps, \
         tc.tile_pool(name="dram", bufs=1, space="DRAM") as dram:

        # ---- constants ----
        ones = const.tile([P, H], F32)
        nc.vector.memset(ones[:], 1.0)
        io = const.tile([P, P], I32)
        nc.gpsimd.iota(io[:], [[1, P]], base=0, channel_multiplier=-1)
        ident = const.tile([P, P], F32)
        nc.vector.tensor_scalar(ident[:], io[:], 0, None, ALU.is_equal)
        ones1 = const.tile([1, P], F32)
        nc.vector.memset(ones1[:], 1.0)

        # ---- DRAM bounce buffers ----
        # wire dtype bf16 when that image's values are bf16-exact. When both
        # images share a dtype, one stacked AllToAll (fewer collectives =
        # less latency exposure); otherwise one per image, issued as soon as
        # that image's blocks are staged.
        dts = (BF16 if bf_gt else F32, BF16 if bf_pred else F32)
        same_dt = dts[0] == dts[1]
        if same_dt:
            a2a_in = [dram.tile([2 * H, P], dts[0], name="a2ai",
                                tag="a2ai")] * 2
            a2a_out = [dram.tile([2 * H, P], dts[0], name="a2ao",
                                 tag="a2ao")] * 2
        else:
            a2a_in = [dram.tile([H, P], dts[m], name=f"a2ai{m}",
                                tag=f"a2ai{m}") for m in range(2)]
            a2a_out = [dram.tile([H, P], dts[m], name=f"a2ao{m}",
                                 tag=f"a2ao{m}") for m in range(2)]
        ar_in = dram.tile([1, 8], F32)
        ar_out = nc.dram_tensor("ar_out_sh", [1, 8], F32, addr_space="Shared")

        # ---- warm-up collective ----
        # The first collective of an execution pays a ~80us latency floor in
        # this runtime; later ones cost ~15-30us. Fire a dummy AllReduce at
        # t=0 so the floor overlaps the row pass instead of serializing
        # before the AllToAll. Its (zero) output is max-folded into the real
        # max partials, which keeps it live and is mathematically a no-op.
        warm_in = dram.tile([1, 8], F32)
        warm_out = nc.dram_tensor("warm_out_sh", [1, 8], F32,
                                  addr_space="Shared")
        wz = work.tile([1, 8], F32, tag="wz")
        nc.vector.memset(wz[:], 0.0)
        nc.sync.dma_start(warm_in[:, :], wz[:])
        nc.gpsimd.collective_compute(
            "AllReduce", ALU.max, replica_groups=rg,
            ins=[warm_in[:, :].opt()], outs=[warm_out[:, :].opt()])

        # ================= phase 1: row pass =================
        for m, (src, w) in enumerate(((gt_rows, w_gt), (pred_rows, w_pred))):
            x = work.tile([P, H], F32, tag="x")
            for q in range(4):  # chunked input DMA -> parallel queues
                nc.sync.dma_start(x[q * 32:(q + 1) * 32, :],
                                  src[q * 32:(q + 1) * 32, :])
            z = work.tile([P, H], F32, tag="z")
            if m == 0:
                # gt is exactly 0/1: foreground (nonzero) -> INF, bg -> 0
                nc.vector.tensor_scalar_mul(z[:], x[:], INF)
            else:
                # foreground = sigmoid(pred) > 0.5  <=>  pred > 0
                nc.vector.tensor_scalar(z[:], x[:], 0.0, INF, ALU.is_gt,
                                        ALU.mult)
            dl = work.tile([P, H], F32, tag="dl")
            nc.vector.tensor_tensor_scan(dl[:], ones[:], z[:], INF, ALU.add,
                                         ALU.min)
            dr = work.tile([P, H], F32, tag="dr")
            nc.vector.tensor_tensor_scan(dr[:, ::-1], ones[:], z[:, ::-1],
                                         INF, ALU.add, ALU.min)
            g = work.tile([P, H], F32, tag="g")
            nc.vector.tensor_tensor(g[:], dl[:], dr[:], ALU.min)
            if w >= H - 1:
                gc = work.tile([P, H], F32, tag="gc")
                nc.vector.tensor_scalar_min(gc[:], g[:], BIG)
                g = gc
            g2 = work.tile([P, H], F32, tag="g2")
            nc.scalar.activation(g2[:], g[:], AF.Square)
            for s in range(NCORES):
                pt = ps.tile([P, P], F32, tag="pt", bufs=4)
                nc.tensor.transpose(pt[:], g2[:, s * P:(s + 1) * P], ident[:])
                st = work.tile([P, P], dts[m], tag=f"st{m}")
                nc.scalar.copy(st[:], pt[:])
                base = (s * 2 * P + m * P) if same_dt else s * P
                nc.sync.dma_start(a2a_in[m][base:base + P, :], st[:])
            if not same_dt:
                # exchange this image's blocks while the other one computes
                nc.gpsimd.collective_compute(
                    "AllToAll", ALU.bypass, replica_groups=rg,
                    ins=[a2a_in[m][:, :].opt()],
                    outs=[a2a_out[m][:, :].opt()])
        if same_dt:
            nc.gpsimd.collective_compute(
                "AllToAll", ALU.bypass, replica_groups=rg,
                ins=[a2a_in[0][:, :].opt()], outs=[a2a_out[0][:, :].opt()])

        # ============ phase 3: column min-plus + per-image max ============
        mx12 = work.tile([P, 2], F32, tag="mx12")
        accs = []
        for m, (w, bf) in enumerate(((w_gt, bf_gt), (w_pred, bf_pred))):
            if same_dt:
                bases = [r * 2 * P + m * P for r in range(NCORES)]
            else:
                bases = [r * P for r in range(NCORES)]
            acc = _col_pass(tc, m, w, bf, a2a_out[m], bases, persist, work)
            accs.append(acc)
            nc.vector.reduce_max(mx12[:, m:m + 1], acc[:], axis=AX.X)

        # ================= phase 4: global max =================
        # partition-dim max via PE transpose [128,2] -> [2,128], then a free-
        # dim reduce; the warm-up AllReduce's (zero) output is DMA'd into the
        # spare lanes of ar_in to keep it live.
        pmx = ps.tile([2, P], F32, tag="pmx")
        nc.tensor.transpose(pmx[:], mx12[:], ident[:])
        mxr = work.tile([2, 1], F32, tag="mxr")
        nc.vector.reduce_max(mxr[:], pmx[:], axis=AX.X)
        nc.sync.dma_start(ar_in[0:1, 0:2], mxr[:])
        wback = work.tile([1, 6], F32, tag="wback")
        nc.sync.dma_start(wback[:], warm_out[0:1, 0:6])
        nc.sync.dma_start(ar_in[0:1, 2:8], wback[:])
        nc.gpsimd.collective_compute(
            "AllReduce", ALU.max, replica_groups=rg,
            ins=[ar_in[:, :].opt()], outs=[ar_out[:, :].opt()])
        gmx = work.tile([1, 2], F32, tag="gmx")
        nc.sync.dma_start(gmx[:], ar_out[0:1, 0:2])

        msq = work.tile([1, 2], F32, tag="msq")
        nc.scalar.activation(msq[:], gmx[:], AF.Sqrt)
        m1 = work.tile([1, 2], F32, tag="m1")
        nc.vector.tensor_scalar_add(m1[:], msq[:], 1e-6)
        inv = work.tile([1, 2], F32, tag="inv")
        nc.vector.reciprocal(inv[:], m1[:])
        # broadcast inv across partitions via PE: [128,2] = ones @ inv
        pb = ps.tile([P, 2], F32, tag="pb")
        nc.tensor.matmul(pb[:], ones1[:], inv[:])
        invb = work.tile([P, 2], F32, tag="invb")
        nc.scalar.copy(invb[:], pb[:])

        # ================= phase 5: normalize + masked mean =================
        avals = []
        masks = []
        for m in range(2):
            y = persist.tile([P, H], F32, tag=f"y{m}")
            nc.scalar.activation(y[:], accs[m][:], AF.Sqrt)
            a = persist.tile([P, H], F32, tag=f"a{m}")
            nc.vector.tensor_scalar(a[:], y[:], invb[:, m:m + 1], None,
                                    ALU.mult)
            mk = persist.tile([P, H], F32, tag=f"mk{m}")
            nc.vector.tensor_scalar(mk[:], a[:], 0.1, None, ALU.is_lt)
            avals.append(a)
            masks.append(mk)
        mk = work.tile([P, H], F32, tag="mku")
        nc.vector.tensor_tensor(mk[:], masks[0][:], masks[1][:], ALU.max)
        d = work.tile([P, H], F32, tag="d")
        nc.vector.tensor_sub(d[:], avals[0][:], avals[1][:])
        da = work.tile([P, H], F32, tag="da")
        nc.scalar.activation(da[:], d[:], AF.Abs)
        nc.vector.tensor_tensor(d[:], da[:], mk[:], ALU.mult)
        s12 = work.tile([P, 2], F32, tag="s12")
        nc.vector.reduce_sum(s12[:, 0:1], d[:], axis=AX.X)
        nc.vector.reduce_sum(s12[:, 1:2], mk[:], axis=AX.X)
        # partition-dim sum via PE: [1,2] = ones[128,1]^T @ s12[128,2]
        pv = ps.tile([1, 2], F32, tag="pv")
        nc.tensor.matmul(pv[:], ones[:, 0:1], s12[:])
        pvs = work.tile([1, 2], F32, tag="pvs")
        nc.scalar.copy(pvs[:], pv[:])
        nc.sync.dma_start(partials[:, :], pvs[:])


def _build(w_gt, w_pred, bf_gt, bf_pred):
    nc = bacc.Bacc("TRN2", target_bir_lowering=False, debug=False,
                   num_devices=NCORES)
    gt_rows = nc.dram_tensor("gt_rows", [P, H], F32, kind="ExternalInput")
    pred_rows = nc.dram_tensor("pred_rows", [P, H], F32, kind="ExternalInput")
    partials = nc.dram_tensor("partials", [1, 2], F32, kind="ExternalOutput")
    with tile.TileContext(nc) as tc:
        _body(tc, w_gt, w_pred, bf_gt, bf_pred, gt_rows, pred_rows, partials)
    nc.compile()
    return nc


_PROGRAMS = {}


def _program(*key):
    if key not in _PROGRAMS:
        _PROGRAMS[key] = _build(*key)
    return _PROGRAMS[key]


def _row_gmax(fg):
    """Max over pixels of the in-row distance to the nearest background
    pixel (clamped to BIG). This equals the exact column-pass window bound."""
    idx = np.arange(fg.shape[1], dtype=np.float64)
    zero = ~fg
    left = np.maximum.accumulate(np.where(zero, idx, -np.inf), axis=1)
    right = np.minimum.accumulate(np.where(zero, idx, np.inf)[:, ::-1],
                                  axis=1)[:, ::-1]
    g = np.minimum(np.minimum(idx - left, right - idx), BIG)
    return float(g.max())


def _bucket(gmax):
    need = min(int(np.ceil(gmax)), H - 1)
    for b in _BUCKETS:
        if b >= need:
            return b
    return H - 1


def _run(pred, gt, trace=False):
    pred = np.ascontiguousarray(np.asarray(pred), dtype=np.float32)
    gt = np.ascontiguousarray(np.asarray(gt), dtype=np.float32)
    assert pred.shape == (H, H) and gt.shape == (H, H)
    gm_gt = _row_gmax(gt != 0)
    gm_pred = _row_gmax(pred > 0)
    w_gt, w_pred = _bucket(gm_gt), _bucket(gm_pred)
    bf_gt, bf_pred = gm_gt <= BF16_GMAX, gm_pred <= BF16_GMAX
    nc = _program(w_gt, w_pred, bf_gt, bf_pred)
    in_maps = [{"gt_rows": gt[c * P:(c + 1) * P],
                "pred_rows": pred[c * P:(c + 1) * P]} for c in range(NCORES)]
    res = run_bass_kernel_spmd(nc, in_maps, list(range(NCORES)), trace=trace)
    tot = np.zeros(2, np.float64)
    for r in res.results:
        tot += np.asarray(r["partials"], np.float64).reshape(-1)[:2]
    loss = np.float32(tot[0] / max(tot[1], 1.0))
    return loss, res


def kernel(pred, gt):
    loss, _ = _run(pred, gt)
    return loss



# revision 7
# speedup vs baseline: 1.9119x; 1.9119x over previous
"""BoundaryLoss Trainium2 kernel (8 NeuronCores, SPMD, strip-replicated).

Layout: core c owns output column block [128c, 128c+128). The host hands
each core a strip of every input row covering its block plus a margin of
w columns on each side (w = bucketed max in-row nearest-background
distance, measured exactly on the host as in the previous revision).
Row-local EDT distances never exceed w at the central columns, so each
core can run the full row pass locally — no AllToAll at all, which in the
prior revision serialized ~70us of collective latency ahead of the column
pass.

Pipeline (per core):
  1. Row pass on [128, 8*W] fp16 strips (W = 128+2w; partition p, block b
     holds image row 128b+p). One forward + one reverse
     tensor_tensor_scan per image; the scan chains across block
     boundaries, but any carried-in state reaches a central column with
     value > w and so never wins (margin absorbs it).
  2. PE-transpose the central 128 columns of each block (g, fp16), square
     on the PSUM->SBUF evacuation (ACT), assembling g2^T [128 cols, 1024
     rows] directly — all overlapped with the other image's row pass.
  3. Column min-plus D2[j,i] = min_dd (dd^2 + g2T[j, i+dd]) over
     |dd| <= w on DVE in fp16 when w <= 44 (integers <= 2048 are fp16-
     exact; candidates in (2048, 4096] round by <= 1, a <= 0.05% error),
     f32 (STT pairs) otherwise. Odd shifts read a one-element-shifted
     copy to keep 4-byte alignment for the DVE 2x mode.
  4. Per-image global max via one small AllReduce (a dummy AllReduce at
     t=0 absorbs this runtime's ~55us first-collective barrier under the
     compute), then a short fp16 tail: masks compare unnormalized
     d = sqrt(D2) against 0.1*(max+1e-6), diff/abs/masked partial sums
     with fused accumulate; host sums the 8 partial pairs.
"""
import os
import sys

import numpy as np

for _p in ("/opt/trn_rl_repo", "/root/.axon_site/_ro/trn_rl_repo"):
    if os.path.isdir(_p) and _p not in sys.path:
        sys.path.append(_p)

import concourse.bacc as bacc
import concourse.tile as tile
from concourse import mybir
from concourse.bass_utils import run_bass_kernel_spmd

F32 = mybir.dt.float32
FP16 = mybir.dt.float16
I32 = mybir.dt.int32
AF = mybir.ActivationFunctionType
ALU = mybir.AluOpType
AX = mybir.AxisListType

H = 1024          # image height/width
P = 128           # partitions / rows per block / cols per core block
NB = 8            # row blocks per strip (H / P)
NCORES = 8
BIG = 1.0e4
INF = 1.0e9       # f32 sentinel
HINF = 60000.0    # fp16 sentinel (fp16 max normal is 65504)
FP16_WMAX = 44    # fp16 col pass iff w <= 44 (g^2, dd^2 <= 1936 exact)

_BUCKETS = (8, 10, 12, 14, 16, 18, 20, 22, 24, 26, 28, 32, 36, 40, 44,
            48, 56, 64, 80, 96, 128, 160, 192, 256, 320)


def _col_pass(tc, m, w, gTp, gB, persist, work):
    """Windowed min-plus; returns acc tile [P, H] (fp16 or f32).

    acc[j, i] = min_{|dd| <= w} (dd^2 + gTp[j, w + i + dd]); gTp is
    INF-padded by w on both sides. Entirely on DVE (tensor ops are
    rejected on Pool in this compiler build).
    """
    nc = tc.nc
    use16 = gB is not None

    if use16:
        def shifted(off):  # AP of width H at element offset `off` of gTp
            if off % 2 == 0:
                return gTp[:, off:off + H]
            return gB[:, off - 1:off - 1 + H]
    else:
        def shifted(off):
            return gTp[:, off:off + H]

    acc = persist.tile([P, H], FP16 if use16 else F32, tag=f"acc{m}")
    # Plain TT gets the DVE 2x mode for 16-bit and single-src TS gets 4x,
    # while the fused STT has no fast uop — so for fp16 a 3-op pairwise
    # form beats 2 STTs per dd. dd=1 folds the d=0 term.
    if use16:
        for dd in range(1, w + 1):
            tmp = work.tile([P, H], FP16, tag=f"pm{m}_{dd % 3}")
            nc.vector.tensor_tensor(tmp[:], shifted(w + dd), shifted(w - dd),
                                    ALU.min)
            nc.vector.tensor_scalar_add(tmp[:], tmp[:], float(dd * dd))
            nc.vector.tensor_tensor(
                acc[:], shifted(w) if dd == 1 else acc[:], tmp[:], ALU.min)
    else:
        for dd in range(1, w + 1):
            c = float(dd * dd)
            nc.vector.scalar_tensor_tensor(
                acc[:], shifted(w + dd), c,
                shifted(w) if dd == 1 else acc[:], ALU.add, ALU.min)
            nc.vector.scalar_tensor_tensor(
                acc[:], shifted(w - dd), c, acc[:], ALU.add, ALU.min)
    return acc


def _body(tc, w_gt, w_pred, gts, prs, partials):
    nc = tc.nc
    rg = [list(range(NCORES))]
    ws = (w_gt, w_pred)
    srcs = (gts, prs)
    use16s = tuple(w <= FP16_WMAX for w in ws)

    with tc.tile_pool(name="const", bufs=1) as const, \
         tc.tile_pool(name="work", bufs=2) as work, \
         tc.tile_pool(name="persist", bufs=1) as persist, \
         tc.tile_pool(name="ps", bufs=1, space="PSUM") as ps, \
         tc.tile_pool(name="dram", bufs=1, space="DRAM") as dram:

        # ---- warm-up collective ----
        # The first collective of an execution pays a ~55us barrier +
        # ~10us latency in this runtime; later ones cost ~10us. Fire a
        # dummy AllReduce at t=0 so that floor overlaps the local compute.
        # Its (zero) output is max-folded into the real max partials,
        # which keeps it live and is mathematically a no-op.
        warm_in = dram.tile([1, 8], F32)
        warm_out = nc.dram_tensor("warm_out_sh", [1, 8], F32,
                                  addr_space="Shared")
        wz = work.tile([1, 8], F32, tag="wz")
        nc.vector.memset(wz[:], 0.0)
        nc.sync.dma_start(warm_in[:, :], wz[:])
        nc.gpsimd.collective_compute(
            "AllReduce", ALU.max, replica_groups=rg,
            ins=[warm_in[:, :].opt()], outs=[warm_out[:, :].opt()])

        ar_in = dram.tile([1, 8], F32)
        ar_out = nc.dram_tensor("ar_out_sh", [1, 8], F32, addr_space="Shared")

        # ---- input DMA (both strips in flight immediately) ----
        strips = []
        for m in range(2):
            wd = NB * (P + 2 * ws[m])
            s = persist.tile([P, wd], FP16, tag=f"strip{m}")
            for q in range(4):
                nc.sync.dma_start(s[q * 32:(q + 1) * 32, :],
                                  srcs[m][q * 32:(q + 1) * 32, :])
            strips.append(s)

        # ---- constants (DVE is idle while the strips stream in) ----
        io = const.tile([P, P], I32)
        nc.gpsimd.iota(io[:], [[1, P]], base=0, channel_multiplier=-1)
        ident = const.tile([P, P], F32)
        nc.vector.tensor_scalar(ident[:], io[:], 0, None, ALU.is_equal)
        identh = const.tile([P, P], FP16)
        nc.scalar.copy(identh[:], ident[:])
        ones1 = const.tile([1, P], F32)
        nc.vector.memset(ones1[:], 1.0)
        onesc = const.tile([P, 1], F32)
        nc.vector.memset(onesc[:], 1.0)
        maxwd = max(NB * (P + 2 * w) for w in ws)
        onesh = const.tile([P, maxwd], FP16)
        nc.vector.memset(onesh[:], 1.0)

        # ================= phase 1: row pass =================
        gs = []
        for m in range(2):
            w = ws[m]
            wd = NB * (P + 2 * w)
            s = strips[m]
            # foreground -> HINF, background -> 0. Host pre-scales inputs
            # by 1e30 (saturating fp16) so `> 0` is the fg test for both
            # images and fp16 underflow cannot flip tiny positives.
            z = work.tile([P, wd], FP16, tag=f"z{m}")
            nc.vector.tensor_scalar(z[:], s[:], 0.0, HINF, ALU.is_gt,
                                    ALU.mult)
            dl = work.tile([P, wd], FP16, tag=f"dl{m}")
            nc.vector.tensor_tensor_scan(dl[:], onesh[:, :wd], z[:], INF,
                                         ALU.add, ALU.min)
            dr = work.tile([P, wd], FP16, tag=f"dr{m}")
            nc.vector.tensor_tensor_scan(dr[:, ::-1], onesh[:, :wd],
                                         z[:, ::-1], INF, ALU.add, ALU.min)
            g = work.tile([P, wd], FP16, tag=f"g{m}")
            nc.vector.tensor_tensor(g[:], dl[:], dr[:], ALU.min)
            gs.append(g)

        # ============ phase 2: transpose + square into g2^T ============
        gTps = []
        gBs = []
        for m in range(2):
            w = ws[m]
            use16 = use16s[m]
            dt = FP16 if use16 else F32
            inf = HINF if use16 else INF
            gw = H + 2 * w
            gTp = persist.tile([P, gw], dt, tag=f"gtp{m}")
            nc.vector.memset(gTp[:, :w], inf)
            nc.vector.memset(gTp[:, w + H:], inf)
            W = P + 2 * w
            for b in range(NB):
                pt = ps.tile([P, P], FP16, tag="pt", bufs=4)
                nc.tensor.transpose(pt[:], gs[m][:, b * W + w:b * W + w + P],
                                    identh[:])
                nc.scalar.activation(gTp[:, w + b * P:w + (b + 1) * P], pt[:],
                                     AF.Square)
            if use16:
                # odd shifts read a one-element-shifted copy so the AP
                # stays 4-byte-aligned for the DVE 2x fp16 mode
                gB = persist.tile([P, gw], FP16, tag=f"gb{m}")
                nc.scalar.copy(gB[:, :gw - 1], gTp[:, 1:])
                nc.vector.memset(gB[:, gw - 1:], inf)
            else:
                gB = None
            gTps.append(gTp)
            gBs.append(gB)

        # ============ phase 3: column min-plus + per-image max ============
        mx12 = work.tile([P, 2], F32, tag="mx12")
        ys = []
        for m in range(2):
            acc = _col_pass(tc, m, ws[m], gTps[m], gBs[m], persist, work)
            nc.vector.reduce_max(mx12[:, m:m + 1], acc[:], axis=AX.X)
            # unnormalized distances, precomputed before the AllReduce
            y = persist.tile([P, H], FP16, tag=f"y{m}")
            nc.scalar.activation(y[:], acc[:], AF.Sqrt)
            ys.append(y)

        # ================= phase 4: global max =================
        # partition-dim max via PE transpose [128,2] -> [2,128] + free-dim
        # reduce; the warm-up AllReduce's (zero) output is DMA'd into the
        # spare lanes of ar_in to keep it live.
        pmx = ps.tile([2, P], F32, tag="pmx")
        nc.tensor.transpose(pmx[:], mx12[:], ident[:])
        mxr = work.tile([2, 1], F32, tag="mxr")
        nc.vector.reduce_max(mxr[:], pmx[:], axis=AX.X)
        nc.sync.dma_start(ar_in[0:1, 0:2], mxr[:])
        wback = work.tile([1, 6], F32, tag="wback")
        nc.sync.dma_start(wback[:], warm_out[0:1, 0:6])
        nc.sync.dma_start(ar_in[0:1, 2:8], wback[:])
        nc.gpsimd.collective_compute(
            "AllReduce", ALU.max, replica_groups=rg,
            ins=[ar_in[:, :].opt()], outs=[ar_out[:, :].opt()])
        gmx = work.tile([1, 2], F32, tag="gmx")
        nc.sync.dma_start(gmx[:], ar_out[0:1, 0:2])

        # s4 = [inv0, inv1, thr0, thr1]: inv = 1/(sqrt(max)+1e-6),
        # thr = 0.1*(sqrt(max)+1e-6) (mask on unnormalized d).
        msq = work.tile([1, 4], F32, tag="msq")
        nc.scalar.activation(msq[:, 0:2], gmx[:], AF.Sqrt)
        nc.vector.tensor_scalar_add(msq[:, 2:4], msq[:, 0:2], 1e-6)
        s4 = work.tile([1, 4], F32, tag="s4")
        nc.vector.reciprocal(s4[:, 0:2], msq[:, 2:4])
        nc.vector.tensor_scalar_mul(s4[:, 2:4], msq[:, 2:4], 0.1)
        # broadcast across partitions via PE: [128,4] = ones1^T @ s4
        pb = ps.tile([P, 4], F32, tag="pb")
        nc.tensor.matmul(pb[:], ones1[:], s4[:])
        invb = work.tile([P, 4], F32, tag="invb")
        nc.scalar.copy(invb[:], pb[:])

        # ================= phase 5: normalize + masked mean =================
        a0 = work.tile([P, H], FP16, tag="a0")
        nc.vector.tensor_scalar(a0[:], ys[0][:], invb[:, 0:1], None, ALU.mult)
        a1 = work.tile([P, H], FP16, tag="a1")
        nc.vector.tensor_scalar(a1[:], ys[1][:], invb[:, 1:2], None, ALU.mult)
        df = work.tile([P, H], FP16, tag="df")
        nc.vector.tensor_tensor(df[:], a0[:], a1[:], ALU.subtract)
        da = work.tile([P, H], FP16, tag="da")
        nc.scalar.activation(da[:], df[:], AF.Abs)
        # (a0 < 0.1) | (a1 < 0.1)  ==  min(a0, a1) < 0.1
        mk = work.tile([P, H], FP16, tag="mk")
        nc.vector.tensor_tensor(mk[:], a0[:], a1[:], ALU.min)
        nc.vector.tensor_scalar(mk[:], mk[:], 0.1, None, ALU.is_lt)
        s12 = work.tile([P, 2], F32, tag="s12")
        mdf = work.tile([P, H], FP16, tag="mdf")
        nc.vector.tensor_tensor(mdf[:], da[:], mk[:], ALU.mult)
        nc.vector.reduce_sum(s12[:, 0:1], mdf[:], axis=AX.X)
        nc.vector.reduce_sum(s12[:, 1:2], mk[:], axis=AX.X)
        # partition-dim sum via PE: [1,2] = ones[128,1]^T @ s12[128,2]
        pv = ps.tile([1, 2], F32, tag="pv")
        nc.tensor.matmul(pv[:], onesc[:], s12[:])
        pvs = work.tile([1, 2], F32, tag="pvs")
        nc.scalar.copy(pvs[:], pv[:])
        nc.sync.dma_start(partials[:, :], pvs[:])


def _build(w_gt, w_pred):
    nc = bacc.Bacc("TRN2", target_bir_lowering=False, debug=False,
                   num_devices=NCORES)
    gts = nc.dram_tensor("gts", [P, NB * (P + 2 * w_gt)], FP16,
                         kind="ExternalInput")
    prs = nc.dram_tensor("prs", [P, NB * (P + 2 * w_pred)], FP16,
                         kind="ExternalInput")
    partials = nc.dram_tensor("partials", [1, 2], F32, kind="ExternalOutput")
    with tile.TileContext(nc) as tc:
        _body(tc, w_gt, w_pred, gts, prs, partials)
    nc.compile()
    return nc


_PROGRAMS = {}


def _program(*key):
    if key not in _PROGRAMS:
        _PROGRAMS[key] = _build(*key)
    return _PROGRAMS[key]


def _row_gmax(fg):
    """Max over pixels of the in-row distance to the nearest background
    pixel (clamped to BIG). This equals the exact column-pass window bound."""
    idx = np.arange(fg.shape[1], dtype=np.float64)
    zero = ~fg
    left = np.maximum.accumulate(np.where(zero, idx, -np.inf), axis=1)
    right = np.minimum.accumulate(np.where(zero, idx, np.inf)[:, ::-1],
                                  axis=1)[:, ::-1]
    g = np.minimum(np.minimum(idx - left, right - idx), BIG)
    return float(g.max())


def _bucket(gmax):
    need = min(int(np.ceil(gmax)), H - 1)
    for b in _BUCKETS:
        if b >= need:
            return b
    raise NotImplementedError(
        f"row gmax {gmax} exceeds the supported strip margin {_BUCKETS[-1]}")


def _strips(img, w):
    """Per-core fp16 strips [128, 8*(128+2w)]: strip[c][p, b*(128+2w)+q] =
    scaled img[128*b + p, 128*c - w + q], fg-padded outside the image."""
    x = np.asarray(img, np.float32) * 1e30
    pad = np.full((H, w), np.float32(1e30))
    xp = np.concatenate([pad, x, pad], axis=1)
    W = P + 2 * w
    out = []
    for c in range(NCORES):
        b = xp[:, c * P:c * P + W].astype(np.float16)
        out.append(np.ascontiguousarray(
            b.reshape(NB, P, W).transpose(1, 0, 2).reshape(P, NB * W)))
    return out


def _run(pred, gt, trace=False):
    pred = np.ascontiguousarray(np.asarray(pred), dtype=np.float32)
    gt = np.ascontiguousarray(np.asarray(gt), dtype=np.float32)
    assert pred.shape == (H, H) and gt.shape == (H, H)
    w_gt = _bucket(_row_gmax(gt != 0))
    w_pred = _bucket(_row_gmax(pred > 0))
    nc = _program(w_gt, w_pred)
    sg = _strips(gt, w_gt)
    sp = _strips(pred, w_pred)
    in_maps = [{"gts": sg[c], "prs": sp[c]} for c in range(NCORES)]
    res = run_bass_kernel_spmd(nc, in_maps, list(range(NCORES)), trace=trace)
    tot = np.zeros(2, np.float64)
    for r in res.results:
        tot += np.asarray(r["partials"], np.float64).reshape(-1)[:2]
    loss = np.float32(tot[0] / max(tot[1], 1.0))
    return loss, res


def kernel(pred, gt):
    loss, _ = _run(pred, gt)
    return loss
